# revision 17
# baseline (speedup 1.0000x reference)
"""GCN (3-layer + readout) on 8 Trainium2 NeuronCores.

Strategy (dst-node sharding, 1D graph parallel):
  - Nodes are sharded across 8 cores (6250/core, padded to 6272 = 49 blocks
    of 128).  Each core aggregates messages for the edges whose dst lands in
    its shard.  Self-loops are materialized as explicit edges (i, i): their
    gathered message dinv_i*z_i times the final dst-side dinv_i scale equals
    the reference's implicit dinv^2 self term exactly.
  - Everything on the message path is fp16 (the rel-err budget is 2e-2).
  - Per layer: transform z = h @ W on the PE, scale rows by dinv = deg^-1/2
    (fused into the Scalar-engine PSUM->SBUF copy) so table rows are
    dinv[src]*z[src].  Layer 1's table is computed fully locally by every
    core (x is replicated); layers 2/3 transform the own shard and AllGather
    the shard tables.
  - Edge gathers: one indirect DMA (InstDMACopy SWDGE, int32 row ids, one
    row per partition) per 128-edge chunk.  The ~1.05us Q7 descriptor
    emission per chunk is the hard floor of this kernel (~8ns/edge on the
    Q7 software emission loop; batched InstDMAGatherAnt measures the same
    per-row rate), so everything else is arranged to hide under it.
    fp16 tables halve the gathered bytes vs fp32.  Edges are sorted by src
    row inside each (core, dst-block) bucket for HBM locality.
  - Scatter-add on the TensorEngine with messages stationary:
    psum[64 feat, 128 dst] += msg[128e, 64f]^T @ onehot[128e, 128d], so the
    block aggregate lands feature-major and feeds the next layer's
    transform (lhsT = hT block) with no transposes anywhere.
  - One-hot matrices are static per graph: precomputed on the host, stored
    e-major in DRAM, and streamed per 16-chunk group with a single
    contiguous HWDGE DMA (4KB per partition) instead of being built on the
    Vector engine.
  - dst-side dinv scale happens in feature-major space via a precomputed
    broadcast tile dinvb[64, PADS] (rank-1 PE matmuls of ones x dinv row).
  - Host-side preprocessing is strictly index/metadata work (edge bucketing,
    padding, degree counting); all float math runs on device.
"""

import numpy as np

from concourse import bacc, bass, mybir, tile
from concourse.bass_utils import run_bass_kernel_spmd

# ---------------------------------------------------------------- constants
P = 8                      # cores
N = 50000                  # nodes
IN_DIM = 128
HID = 64
OUT_DIM = 10
BLK = 128
G = 16                     # chunks per onehot-stream group

F32 = mybir.dt.float32
F16 = mybir.dt.float16
I32 = mybir.dt.int32

SHARD = N // P
NBLK = (SHARD + BLK - 1) // BLK      # 49
PADS = NBLK * BLK                    # 6272
TBL = P * PADS                       # 50176
NFULL = P * NBLK                     # 392


# ------------------------------------------------------------- host prep
def _preprocess(x, edge_index):
    """Bucket edges (incl. one self-edge per node) into per-(core, dst-block)
    128-edge chunks.

    Nodes are bin-packed into the P*NBLK (core, block) bins by in-degree
    (capacity-constrained LPT) so every bin carries ~the same edge count —
    this minimizes the uniform per-block chunk counts, which set the Q7
    gather-instruction floor.
    """
    import heapq

    x = np.asarray(x, np.float32)
    ei = np.asarray(edge_index, np.int64)
    src, dst = ei[0], ei[1]

    degE = np.bincount(dst, minlength=N).astype(np.int64)
    deg = (degE + 1).astype(np.float32)

    NBINS = P * NBLK
    order_n = np.argsort(-degE, kind="stable")
    heap = [(0, b) for b in range(NBINS)]
    heapq.heapify(heap)
    fill = np.zeros(NBINS, np.int64)
    node_bin = np.empty(N, np.int64)
    node_slot = np.empty(N, np.int64)
    for n in order_n:
        while True:
            s, b = heapq.heappop(heap)
            if fill[b] < BLK:
                break
        node_bin[n] = b
        node_slot[n] = fill[b]
        fill[b] += 1
        heapq.heappush(heap, (s + int(degE[n]) + 1, b))

    newid = node_bin * BLK + node_slot          # padded global row of each node

    # edge stream = input edges + one self edge per node
    all_src = np.concatenate([src, np.arange(N, dtype=np.int64)])
    all_dst = np.concatenate([dst, np.arange(N, dtype=np.int64)])

    rows = newid[all_src]
    owner = node_bin[all_dst] // NBLK
    blk = node_bin[all_dst] % NBLK
    dstl = node_slot[all_dst].astype(np.int64)

    # bucket + in-bucket src sort (HBM locality for the gather descriptors)
    key = owner * NBLK + blk
    order = np.lexsort((rows, key))
    key_s = key[order]
    counts = np.bincount(key_s, minlength=P * NBLK)
    starts = np.concatenate([[0], np.cumsum(counts)[:-1]])
    pos = np.arange(key_s.size) - starts[key_s]

    # per-block chunk count: max over cores (program is core-uniform)
    C_arr = np.maximum(np.ceil(
        counts.reshape(P, NBLK).max(axis=0) / BLK).astype(np.int64), 1)
    base = np.concatenate([[0], np.cumsum(C_arr)[:-1]])
    T = int(C_arr.sum())

    own_s = key_s // NBLK
    blk_s = key_s % NBLK
    slot = base[blk_s] * BLK + pos            # (chunk, lane) within the stream
    flat = own_s * (T * BLK) + slot

    gidx = np.zeros((P, T * BLK), np.int32)
    gidx.reshape(-1)[flat] = rows[order]
    dv = np.full((P, T * BLK), -1, np.int64)
    dv.reshape(-1)[flat] = dstl[order]

    lanes = np.arange(T * BLK)
    x_pad = np.zeros((TBL, IN_DIM), np.float32)
    deg_pad = np.ones((P, PADS), np.float32)
    x_pad[newid] = x
    deg_pad.reshape(-1)[newid] = deg
    xpt = np.ascontiguousarray(x_pad.T.astype(np.float16))          # [128, TBL]
    degp = np.ascontiguousarray(
        deg_pad.reshape(NFULL, BLK).T)                              # [128, 392]

    per_core = []
    for k in range(P):
        oh = np.zeros((BLK, T * BLK), np.float16)
        dvk = dv[k]
        sel = dvk >= 0
        oh[lanes[sel] % BLK, (lanes[sel] // BLK) * BLK + dvk[sel]] = 1.0
        per_core.append(dict(
            xpo=np.ascontiguousarray(xpt[:, k * PADS:(k + 1) * PADS]),
            dego=np.ascontiguousarray(degp[:, k * NBLK:(k + 1) * NBLK]),
            degbt=np.ascontiguousarray(deg_pad[k].reshape(1, PADS)),
            gidx=np.ascontiguousarray(gidx[k].reshape(T, BLK).T),
            ohd=oh,
        ))
    return per_core, tuple(int(c) for c in C_arr), newid


# ------------------------------------------------------------- device build
def _build(C_arr):
    T = int(sum(C_arr))
    c_base = [0]
    for c in C_arr[:-1]:
        c_base.append(c_base[-1] + c)

    nc = bacc.Bacc("TRN2", target_bir_lowering=False, debug=False,
                   enable_asserts=False, num_devices=P,
                   dynamic_dma_scratch_size=65536)

    xpo_d = nc.dram_tensor("xpo", [IN_DIM, PADS], F16, kind="ExternalInput").ap()
    dego_d = nc.dram_tensor("dego", [BLK, NBLK], F32, kind="ExternalInput").ap()
    degbt_d = nc.dram_tensor("degbt", [1, PADS], F32, kind="ExternalInput").ap()
    gidx_d = nc.dram_tensor("gidx", [BLK, T], I32, kind="ExternalInput").ap()
    ohd_d = nc.dram_tensor("ohd", [BLK, T * BLK], F16, kind="ExternalInput").ap()
    w_d = [nc.dram_tensor(f"w{i}", [d, HID if i < 3 else OUT_DIM], F16,
                          kind="ExternalInput").ap()
           for i, d in enumerate([IN_DIM, HID, HID, HID])]
    bc_d = [nc.dram_tensor(f"bc{i}", [HID, 1], F32, kind="ExternalInput").ap()
            for i in range(3)]
    btr_d = nc.dram_tensor("btr", [BLK, OUT_DIM], F32, kind="ExternalInput").ap()
    out_d = nc.dram_tensor("probs", [PADS, OUT_DIM], F32, kind="ExternalOutput").ap()

    rg = [list(range(P))]

    with tile.TileContext(nc) as tc:
        with (
            tc.tile_pool(name="const", bufs=1) as cp,
            tc.tile_pool(name="zt", bufs=3) as zp,
            tc.tile_pool(name="oh", bufs=4) as ohp,
            tc.tile_pool(name="msg", bufs=4) as mp,
            tc.tile_pool(name="cmb", bufs=4) as cb,
            tc.tile_pool(name="fin", bufs=2) as fp,
            tc.tile_pool(name="psz", bufs=3, space="PSUM") as psz,
            tc.tile_pool(name="psacc", bufs=3, space="PSUM") as psacc,
            tc.tile_pool(name="pso", bufs=1, space="PSUM") as pso,
            tc.tile_pool(name="dram", bufs=1, space="DRAM") as dp,
        ):
            # ---- constants into SBUF
            w_sb, bc_sb = [], []
            for i in range(4):
                wt = cp.tile(list(w_d[i].shape), F16, tag=f"w{i}", name=f"w{i}")
                nc.sync.dma_start(wt[:], w_d[i])
                w_sb.append(wt)
            for i in range(3):
                bt = cp.tile([HID, 1], F32, tag=f"bc{i}", name=f"bc{i}")
                nc.sync.dma_start(bt[:], bc_d[i])
                bc_sb.append(bt)
            btr_sb = cp.tile([BLK, OUT_DIM], F32, tag="btr")
            nc.sync.dma_start(btr_sb[:], btr_d)
            gidx_sb = cp.tile([BLK, T], I32, tag="gidx")
            nc.sync.dma_start(gidx_sb[:], gidx_d)
            xpo_sb = cp.tile([IN_DIM, PADS], F16, tag="xpo")
            nc.sync.dma_start(xpo_sb[:], xpo_d)

            # dinv = deg^-1/2 in the two layouts we need
            dinvo_sb = cp.tile([BLK, NBLK], F32, tag="dinvo")
            nc.sync.dma_start(dinvo_sb[:], dego_d)
            nc.vector.reciprocal(dinvo_sb[:], dinvo_sb[:])
            nc.scalar.activation(dinvo_sb[:], dinvo_sb[:],
                                 mybir.ActivationFunctionType.Sqrt)
            dinvbt_sb = cp.tile([1, PADS], F32, tag="dinvbt")
            nc.sync.dma_start(dinvbt_sb[:], degbt_d)
            nc.vector.reciprocal(dinvbt_sb[:], dinvbt_sb[:])
            nc.scalar.activation(dinvbt_sb[:], dinvbt_sb[:],
                                 mybir.ActivationFunctionType.Sqrt)

            # dinvb[64, PADS] (f16): dst-side dinv broadcast across the
            # feature partitions, built with rank-1 matmuls ones^T x dinv_row
            ones_sb = cp.tile([1, HID], F32, tag="ones")
            nc.vector.memset(ones_sb[:], 1.0)
            dinvb_sb = cp.tile([HID, PADS], F16, tag="dinvb")
            for i in range(PADS // 448):
                off = i * 448
                ps = pso.tile([HID, 448], F32, tag="bc", name="bc_ps")
                nc.tensor.matmul(ps[:], ones_sb[:], dinvbt_sb[:, off:off + 448],
                                 start=True, stop=True)
                nc.vector.tensor_copy(dinvb_sb[:, off:off + 448], ps[:])

            hT = [cp.tile([HID, PADS], F16, tag=f"h{i}", name=f"h{i}")
                  for i in range(2)]

            TGB = 7

            def transform_layer(src_sb, li):
                """table = AllGather(dinv * (src @ W_li)) — src feature-major."""
                ag_in = dp.tile([PADS, HID], F16, tag=f"agin{li}",
                                name=f"agin{li}")
                for g in range(NBLK // TGB):
                    zg = zp.tile([BLK, TGB * HID], F16, tag="zd", name="zd")
                    for j in range(TGB):
                        b = g * TGB + j
                        z_ps = psz.tile([BLK, HID], F32, tag="z", name="z_ps")
                        nc.tensor.matmul(z_ps[:],
                                         src_sb[:, b * BLK:(b + 1) * BLK],
                                         w_sb[li][:], start=True, stop=True)
                        nc.scalar.activation(zg[:, j * HID:(j + 1) * HID],
                                             z_ps[:],
                                             mybir.ActivationFunctionType.Copy,
                                             scale=dinvo_sb[:, b:b + 1])
                    nc.sync.dma_start(
                        ag_in[g * TGB * BLK:(g + 1) * TGB * BLK, :].rearrange(
                            "(j p) f -> p j f", p=BLK),
                        zg[:].rearrange("p (j f) -> p j f", f=HID))
                table = dp.tile([TBL, HID], F16, tag=f"tbl{li}",
                                name=f"table{li}", addr_space="Shared")
                nc.gpsimd.collective_compute(
                    "AllGather", mybir.AluOpType.bypass, replica_groups=rg,
                    ins=[ag_in.opt()], outs=[table.opt()])
                return table

            def readout_block(h_ap, b):
                o_ps = pso.tile([BLK, OUT_DIM], F32, tag="o", name="o_ps")
                nc.tensor.matmul(o_ps[:], h_ap, w_sb[3][:], start=True, stop=True)
                logit = fp.tile([BLK, OUT_DIM], F32, tag="logit", name="logit")
                nc.vector.tensor_tensor(logit[:], o_ps[:], btr_sb[:],
                                        mybir.AluOpType.add)
                nmx = fp.tile([BLK, 1], F32, tag="nmx", name="nmx")
                nc.vector.reduce_max(nmx[:], logit[:],
                                     axis=mybir.AxisListType.X, negate=True)
                ex = fp.tile([BLK, OUT_DIM], F32, tag="ex", name="ex")
                ssum = fp.tile([BLK, 1], F32, tag="ssum", name="ssum")
                nc.scalar.activation(ex[:], logit[:],
                                     mybir.ActivationFunctionType.Exp,
                                     bias=nmx[:], accum_out=ssum[:])
                rs = fp.tile([BLK, 1], F32, tag="rs", name="rs")
                nc.vector.reciprocal(rs[:], ssum[:])
                prob = fp.tile([BLK, OUT_DIM], F32, tag="prob", name="prob")
                nc.vector.tensor_scalar(prob[:], ex[:], rs[:], None,
                                        mybir.AluOpType.mult)
                nc.sync.dma_start(out_d[b * BLK:(b + 1) * BLK, :], prob[:])

            def propagate(table, h_out, bc_t, readout=False):
                tiles = {}

                def group(tg):
                    if tg not in tiles:
                        lo = tg * G
                        n = min(T, lo + G) - lo
                        oh = ohp.tile([BLK, G * BLK], F16, tag="oh", name="oh")
                        nc.sync.dma_start(oh[:, :n * BLK],
                                          ohd_d[:, lo * BLK:(lo + n) * BLK])
                        mt = mp.tile([BLK, G * HID], F16, tag="msg", name="msg")
                        for c2 in range(n):
                            nc.gpsimd.indirect_dma_start(
                                out=mt[:, c2 * HID:(c2 + 1) * HID],
                                out_offset=None, in_=table[:],
                                in_offset=bass.IndirectOffsetOnAxis(
                                    ap=gidx_sb[:, lo + c2:lo + c2 + 1], axis=0))
                        tiles[tg] = (mt, oh)
                    return tiles[tg]

                for b in range(NBLK):
                    C_b = C_arr[b]
                    agg = psacc.tile([HID, BLK], F32, tag="acc", name="agg")
                    for c in range(C_b):
                        t = c_base[b] + c
                        tg, r = divmod(t, G)
                        mt, oh = group(tg)
                        nc.tensor.matmul(agg[:],
                                         mt[:, r * HID:(r + 1) * HID],
                                         oh[:, r * BLK:(r + 1) * BLK],
                                         start=(c == 0), stop=(c == C_b - 1))
                    sl = slice(b * BLK, (b + 1) * BLK)
                    tmp = cb.tile([HID, BLK], F16, tag="tmp", name="tmp")
                    nc.vector.tensor_tensor(tmp[:], agg[:], dinvb_sb[:, sl],
                                            mybir.AluOpType.mult)
                    nc.scalar.activation(h_out[:, sl], tmp[:],
                                         mybir.ActivationFunctionType.Relu,
                                         bias=bc_t[:])
                    if readout:
                        readout_block(h_out[:, sl], b)

            # layer 1 transforms the own x shard; layers 2/3 the own h shard
            table = transform_layer(xpo_sb, 0)
            propagate(table, hT[0], bc_sb[0])
            for li in (1, 2):
                table = transform_layer(hT[(li + 1) % 2], li)
                propagate(table, hT[li % 2], bc_sb[li], readout=(li == 2))

    nc.compile()
    return nc


# ------------------------------------------------------------- entry point
_CACHE = {}


def _get_program(C_arr):
    if C_arr not in _CACHE:
        _CACHE[C_arr] = _build(C_arr)
    return _CACHE[C_arr]


def kernel(x, edge_index, W1, b1, W2, b2, W3, b3, Wr, br, trace=False):
    per_core, C_arr, newid = _preprocess(x, edge_index)
    nc = _get_program(C_arr)

    ws = [np.asarray(w, np.float16) for w in (W1, W2, W3, Wr)]
    bcs = [np.asarray(b, np.float32).reshape(HID, 1) for b in (b1, b2, b3)]
    btr = np.tile(np.asarray(br, np.float32).reshape(1, -1), (BLK, 1))

    in_maps = []
    for k in range(P):
        m = dict(per_core[k])
        for i in range(4):
            m[f"w{i}"] = ws[i]
        for i in range(3):
            m[f"bc{i}"] = bcs[i]
        m["btr"] = btr
        in_maps.append(m)

    res = run_bass_kernel_spmd(nc, in_maps, core_ids=list(range(P)),
                               trace=trace)
    allp = np.concatenate([res.results[k]["probs"] for k in range(P)], axis=0)
    out = allp[newid]
    kernel.last_results = res
    return out


# revision 25
# speedup vs baseline: 1.2422x; 1.2422x over previous
"""GCN (3-layer + readout) on 8 Trainium2 NeuronCores.

Strategy (dst-node sharding, 1D graph parallel):
  - Nodes are sharded across 8 cores (6250/core, padded to 6272 = 49 blocks
    of 128).  Each core aggregates messages for the edges whose dst lands in
    its shard.  Self-loops are materialized as explicit edges (i, i): their
    gathered message dinv_i*z_i times the final dst-side dinv_i scale equals
    the reference's implicit dinv^2 self term exactly.
  - Everything on the message path is fp16 (the rel-err budget is 2e-2).
  - Per layer: transform z = h @ W on the PE, scale rows by dinv = deg^-1/2
    (fused into the Scalar-engine PSUM->SBUF copy) so table rows are
    dinv[src]*z[src].  Layer 1's table is computed fully locally by every
    core (x is replicated); layers 2/3 transform the own shard and AllGather
    the shard tables.
  - Edge gathers: one indirect DMA (InstDMACopy SWDGE, int32 row ids, one
    row per partition) per 128-edge chunk.  The ~1.05us Q7 descriptor
    emission per chunk is the hard floor of this kernel (~8ns/edge on the
    Q7 software emission loop; batched InstDMAGatherAnt measures the same
    per-row rate), so everything else is arranged to hide under it.
    fp16 tables halve the gathered bytes vs fp32.  Edges are sorted by src
    row inside each (core, dst-block) bucket for HBM locality.
  - Scatter-add on the TensorEngine with messages stationary:
    psum[64 feat, 128 dst] += msg[128e, 64f]^T @ onehot[128e, 128d], so the
    block aggregate lands feature-major and feeds the next layer's
    transform (lhsT = hT block) with no transposes anywhere.
  - One-hot matrices are static per graph: precomputed on the host, stored
    e-major in DRAM, and streamed per 16-chunk group with a single
    contiguous HWDGE DMA (4KB per partition) instead of being built on the
    Vector engine.
  - dst-side dinv scale happens in feature-major space via a precomputed
    broadcast tile dinvb[64, PADS] (rank-1 PE matmuls of ones x dinv row).
  - Host-side preprocessing is strictly index/metadata work (edge bucketing,
    padding, degree counting); all float math runs on device.
"""

import numpy as np

from concourse import bacc, bass, mybir, tile
from concourse.bass_utils import run_bass_kernel_spmd

# ---------------------------------------------------------------- constants
P = 8                      # cores
N = 50000                  # nodes
IN_DIM = 128
HID = 64
OUT_DIM = 10
BLK = 128
G = 16                     # chunks per onehot-stream group

F32 = mybir.dt.float32
F16 = mybir.dt.float16
I32 = mybir.dt.int32

SHARD = N // P
NBLK = (SHARD + BLK - 1) // BLK      # 49
PADS = NBLK * BLK                    # 6272
TBL = P * PADS                       # 50176
NFULL = P * NBLK                     # 392


# ------------------------------------------------------------- host prep
def _preprocess(x, edge_index):
    """Bucket edges (incl. one self-edge per node) into per-(core, dst-block)
    128-edge chunks.

    Nodes are bin-packed into the P*NBLK (core, block) bins by in-degree
    (capacity-constrained LPT) so every bin carries ~the same edge count —
    this minimizes the uniform per-block chunk counts, which set the Q7
    gather-instruction floor.
    """
    import heapq

    x = np.asarray(x, np.float32)
    ei = np.asarray(edge_index, np.int64)
    src, dst = ei[0], ei[1]

    degE = np.bincount(dst, minlength=N).astype(np.int64)
    deg = (degE + 1).astype(np.float32)

    NBINS = P * NBLK
    order_n = np.argsort(-degE, kind="stable")
    heap = [(0, b) for b in range(NBINS)]
    heapq.heapify(heap)
    fill = np.zeros(NBINS, np.int64)
    node_bin = np.empty(N, np.int64)
    node_slot = np.empty(N, np.int64)
    for n in order_n:
        while True:
            s, b = heapq.heappop(heap)
            if fill[b] < BLK:
                break
        node_bin[n] = b
        node_slot[n] = fill[b]
        fill[b] += 1
        heapq.heappush(heap, (s + int(degE[n]) + 1, b))

    newid = node_bin * BLK + node_slot          # padded global row of each node

    # self-loops are handled as an explicit dinv^2*z term, not as edges
    all_src, all_dst = src, dst

    rows = newid[all_src]
    owner = node_bin[all_dst] // NBLK
    blk = node_bin[all_dst] % NBLK
    dstl = node_slot[all_dst].astype(np.int64)

    # bucket + in-bucket src sort (HBM locality for the gather descriptors)
    key = owner * NBLK + blk
    order = np.lexsort((rows, key))
    key_s = key[order]
    counts = np.bincount(key_s, minlength=P * NBLK)
    starts = np.concatenate([[0], np.cumsum(counts)[:-1]])
    pos = np.arange(key_s.size) - starts[key_s]

    # per-block chunk count: max over cores (program is core-uniform)
    C_arr = np.maximum(np.ceil(
        counts.reshape(P, NBLK).max(axis=0) / BLK).astype(np.int64), 1)
    base = np.concatenate([[0], np.cumsum(C_arr)[:-1]])
    T = int(C_arr.sum())

    own_s = key_s // NBLK
    blk_s = key_s % NBLK
    slot = base[blk_s] * BLK + pos            # (chunk, lane) within the stream
    flat = own_s * (T * BLK) + slot

    gidx = np.zeros((P, T * BLK), np.int32)
    gidx.reshape(-1)[flat] = rows[order]
    dv = np.full((P, T * BLK), -1, np.int64)
    dv.reshape(-1)[flat] = dstl[order]

    lanes = np.arange(T * BLK)
    x_pad = np.zeros((TBL, IN_DIM), np.float32)
    deg_pad = np.ones((P, PADS), np.float32)
    x_pad[newid] = x
    deg_pad.reshape(-1)[newid] = deg
    xpt = np.ascontiguousarray(x_pad.T.astype(np.float16))          # [128, TBL]
    degp = np.ascontiguousarray(
        deg_pad.reshape(NFULL, BLK).T)                              # [128, 392]

    per_core = []
    for k in range(P):
        oh = np.zeros((BLK, T * BLK), np.float16)
        dvk = dv[k]
        sel = dvk >= 0
        oh[lanes[sel] % BLK, (lanes[sel] // BLK) * BLK + dvk[sel]] = 1.0
        per_core.append(dict(
            xpo=np.ascontiguousarray(xpt[:, k * PADS:(k + 1) * PADS]),
            dego=np.ascontiguousarray(degp[:, k * NBLK:(k + 1) * NBLK]),
            degbt=np.ascontiguousarray(deg_pad[k].reshape(1, PADS)),
            gidx=np.ascontiguousarray(gidx[k].reshape(T, BLK).T),
            ohd=oh,
            iden=np.eye(BLK, dtype=np.float16),
        ))
    return per_core, tuple(int(c) for c in C_arr), newid


# ------------------------------------------------------------- device build
def _build(C_arr):
    T = int(sum(C_arr))
    c_base = [0]
    for c in C_arr[:-1]:
        c_base.append(c_base[-1] + c)

    nc = bacc.Bacc("TRN2", target_bir_lowering=False, debug=False,
                   enable_asserts=False, num_devices=P,
                   dynamic_dma_scratch_size=65536)

    xpo_d = nc.dram_tensor("xpo", [IN_DIM, PADS], F16, kind="ExternalInput").ap()
    dego_d = nc.dram_tensor("dego", [BLK, NBLK], F32, kind="ExternalInput").ap()
    degbt_d = nc.dram_tensor("degbt", [1, PADS], F32, kind="ExternalInput").ap()
    gidx_d = nc.dram_tensor("gidx", [BLK, T], I32, kind="ExternalInput").ap()
    ohd_d = nc.dram_tensor("ohd", [BLK, T * BLK], F16, kind="ExternalInput").ap()
    w_d = [nc.dram_tensor(f"w{i}", [d, HID if i < 3 else OUT_DIM], F16,
                          kind="ExternalInput").ap()
           for i, d in enumerate([IN_DIM, HID, HID, HID])]
    bc_d = [nc.dram_tensor(f"bc{i}", [HID, 1], F32, kind="ExternalInput").ap()
            for i in range(3)]
    btr_d = nc.dram_tensor("btr", [BLK, OUT_DIM], F32, kind="ExternalInput").ap()
    iden_d = nc.dram_tensor("iden", [BLK, BLK], F16, kind="ExternalInput").ap()
    out_d = nc.dram_tensor("probs", [PADS, OUT_DIM], F32, kind="ExternalOutput").ap()

    rg = [list(range(P))]

    with tile.TileContext(nc) as tc:
        with (
            tc.tile_pool(name="const", bufs=1) as cp,
            tc.tile_pool(name="zt", bufs=3) as zp,
            tc.tile_pool(name="zT2", bufs=2) as ztp,
            tc.tile_pool(name="oh", bufs=4) as ohp,
            tc.tile_pool(name="msg", bufs=24) as mp,
            tc.tile_pool(name="cmb", bufs=4) as cb,
            tc.tile_pool(name="fin", bufs=2) as fp,
            tc.tile_pool(name="psz", bufs=2, space="PSUM") as psz,
            tc.tile_pool(name="pstp", bufs=1, space="PSUM") as pstp,
            tc.tile_pool(name="psacc", bufs=3, space="PSUM") as psacc,
            tc.tile_pool(name="pso", bufs=1, space="PSUM") as pso,
            tc.tile_pool(name="dram", bufs=1, space="DRAM") as dp,
        ):
            # ---- constants into SBUF
            w_sb, bc_sb = [], []
            for i in range(4):
                wt = cp.tile(list(w_d[i].shape), F16, tag=f"w{i}", name=f"w{i}")
                nc.sync.dma_start(wt[:], w_d[i])
                w_sb.append(wt)
            for i in range(3):
                bt = cp.tile([HID, 1], F32, tag=f"bc{i}", name=f"bc{i}")
                nc.sync.dma_start(bt[:], bc_d[i])
                bc_sb.append(bt)
            btr_sb = cp.tile([BLK, OUT_DIM], F32, tag="btr")
            nc.sync.dma_start(btr_sb[:], btr_d)
            iden_sb = cp.tile([BLK, BLK], F16, tag="iden")
            nc.sync.dma_start(iden_sb[:], iden_d)
            gidx_sb = cp.tile([BLK, T], I32, tag="gidx")
            nc.sync.dma_start(gidx_sb[:], gidx_d)
            xpo_sb = cp.tile([IN_DIM, PADS], F16, tag="xpo")
            nc.sync.dma_start(xpo_sb[:], xpo_d)

            # dinv = deg^-1/2 in the two layouts we need
            dinvo_sb = cp.tile([BLK, NBLK], F32, tag="dinvo")
            nc.sync.dma_start(dinvo_sb[:], dego_d)
            nc.vector.reciprocal(dinvo_sb[:], dinvo_sb[:])
            nc.scalar.activation(dinvo_sb[:], dinvo_sb[:],
                                 mybir.ActivationFunctionType.Sqrt)
            dinvbt_sb = cp.tile([1, PADS], F32, tag="dinvbt")
            nc.sync.dma_start(dinvbt_sb[:], degbt_d)
            nc.vector.reciprocal(dinvbt_sb[:], dinvbt_sb[:])
            nc.scalar.activation(dinvbt_sb[:], dinvbt_sb[:],
                                 mybir.ActivationFunctionType.Sqrt)

            # dinvb[64, PADS] (f16): dst-side dinv broadcast across the
            # feature partitions, built with rank-1 matmuls ones^T x dinv_row
            ones_sb = cp.tile([1, HID], F32, tag="ones")
            nc.vector.memset(ones_sb[:], 1.0)
            dinvb_sb = cp.tile([HID, PADS], F16, tag="dinvb")
            for i in range(PADS // 448):
                off = i * 448
                ps = pso.tile([HID, 448], F32, tag="bc", name="bc_ps")
                nc.tensor.matmul(ps[:], ones_sb[:], dinvbt_sb[:, off:off + 448],
                                 start=True, stop=True)
                nc.vector.tensor_copy(dinvb_sb[:, off:off + 448], ps[:])

            hT = [cp.tile([HID, PADS], F16, tag=f"h{i}", name=f"h{i}")
                  for i in range(2)]

            TGB = 7

            def transform_layer(src_sb, li):
                """table = AllGather(dinv * (src @ W_li)) — src feature-major.

                Also returns zT[64, PADS], the own-shard table rows
                transposed back to feature-major for the self-loop term."""
                ag_in = dp.tile([PADS, HID], F16, tag=f"agin{li}",
                                name=f"agin{li}")
                zT = ztp.tile([HID, PADS], F16, tag="zT", name="zT")
                for g in range(NBLK // TGB):
                    zg = zp.tile([BLK, TGB * HID], F16, tag="zd", name="zd")
                    for j in range(TGB):
                        b = g * TGB + j
                        z_ps = psz.tile([BLK, HID], F32, tag="z", name="z_ps")
                        nc.tensor.matmul(z_ps[:],
                                         src_sb[:, b * BLK:(b + 1) * BLK],
                                         w_sb[li][:], start=True, stop=True)
                        nc.scalar.activation(zg[:, j * HID:(j + 1) * HID],
                                             z_ps[:],
                                             mybir.ActivationFunctionType.Copy,
                                             scale=dinvo_sb[:, b:b + 1])
                        tp = pstp.tile([HID, BLK], F16, tag="tp", name="tp")
                        nc.tensor.transpose(tp[:],
                                            zg[:, j * HID:(j + 1) * HID],
                                            iden_sb[:])
                        nc.vector.tensor_copy(zT[:, b * BLK:(b + 1) * BLK],
                                              tp[:])
                    nc.sync.dma_start(
                        ag_in[g * TGB * BLK:(g + 1) * TGB * BLK, :].rearrange(
                            "(j p) f -> p j f", p=BLK),
                        zg[:].rearrange("p (j f) -> p j f", f=HID))
                table = dp.tile([TBL, HID], F16, tag=f"tbl{li}",
                                name=f"table{li}", addr_space="Shared")
                nc.gpsimd.collective_compute(
                    "AllGather", mybir.AluOpType.bypass, replica_groups=rg,
                    ins=[ag_in.opt()], outs=[table.opt()])
                return table, zT

            def readout_block(h_ap, b):
                o_ps = pso.tile([BLK, OUT_DIM], F32, tag="o", name="o_ps")
                nc.tensor.matmul(o_ps[:], h_ap, w_sb[3][:], start=True, stop=True)
                logit = fp.tile([BLK, OUT_DIM], F32, tag="logit", name="logit")
                nc.vector.tensor_tensor(logit[:], o_ps[:], btr_sb[:],
                                        mybir.AluOpType.add)
                nmx = fp.tile([BLK, 1], F32, tag="nmx", name="nmx")
                nc.vector.reduce_max(nmx[:], logit[:],
                                     axis=mybir.AxisListType.X, negate=True)
                ex = fp.tile([BLK, OUT_DIM], F32, tag="ex", name="ex")
                ssum = fp.tile([BLK, 1], F32, tag="ssum", name="ssum")
                nc.scalar.activation(ex[:], logit[:],
                                     mybir.ActivationFunctionType.Exp,
                                     bias=nmx[:], accum_out=ssum[:])
                rs = fp.tile([BLK, 1], F32, tag="rs", name="rs")
                nc.vector.reciprocal(rs[:], ssum[:])
                prob = fp.tile([BLK, OUT_DIM], F32, tag="prob", name="prob")
                nc.vector.tensor_scalar(prob[:], ex[:], rs[:], None,
                                        mybir.AluOpType.mult)
                nc.sync.dma_start(out_d[b * BLK:(b + 1) * BLK, :], prob[:])

            def propagate(table, zT, h_out, bc_t, readout=False):
                oh_tiles = {}

                def oh_group(tg):
                    if tg not in oh_tiles:
                        lo = tg * G
                        n = min(T, lo + G) - lo
                        oh = ohp.tile([BLK, G * BLK], F16, tag="oh", name="oh")
                        nc.sync.dma_start(oh[:, :n * BLK],
                                          ohd_d[:, lo * BLK:(lo + n) * BLK])
                        oh_tiles[tg] = oh
                    return oh_tiles[tg]

                for b in range(NBLK):
                    C_b = C_arr[b]
                    agg = psacc.tile([HID, BLK], F32, tag="acc", name="agg")
                    for c in range(C_b):
                        t = c_base[b] + c
                        tg, r = divmod(t, G)
                        oh = oh_group(tg)
                        msg = mp.tile([BLK, HID], F16, tag="msg", name="msg")
                        nc.gpsimd.indirect_dma_start(
                            out=msg[:], out_offset=None, in_=table[:],
                            in_offset=bass.IndirectOffsetOnAxis(
                                ap=gidx_sb[:, t:t + 1], axis=0))
                        nc.tensor.matmul(agg[:], msg[:],
                                         oh[:, r * BLK:(r + 1) * BLK],
                                         start=(c == 0), stop=(c == C_b - 1))
                    sl = slice(b * BLK, (b + 1) * BLK)
                    t1 = cb.tile([HID, BLK], F16, tag="t1", name="t1")
                    nc.vector.tensor_tensor(t1[:], agg[:], zT[:, sl],
                                            mybir.AluOpType.add)
                    tmp = cb.tile([HID, BLK], F16, tag="tmp", name="tmp")
                    nc.vector.tensor_tensor(tmp[:], t1[:], dinvb_sb[:, sl],
                                            mybir.AluOpType.mult)
                    nc.scalar.activation(h_out[:, sl], tmp[:],
                                         mybir.ActivationFunctionType.Relu,
                                         bias=bc_t[:])
                    if readout:
                        readout_block(h_out[:, sl], b)

            # layer 1 transforms the own x shard; layers 2/3 the own h shard
            table, zT = transform_layer(xpo_sb, 0)
            propagate(table, zT, hT[0], bc_sb[0])
            for li in (1, 2):
                table, zT = transform_layer(hT[(li + 1) % 2], li)
                propagate(table, zT, hT[li % 2], bc_sb[li],
                          readout=(li == 2))

    nc.compile()
    return nc


# ------------------------------------------------------------- entry point
_CACHE = {}


def _get_program(C_arr):
    if C_arr not in _CACHE:
        _CACHE[C_arr] = _build(C_arr)
    return _CACHE[C_arr]


def kernel(x, edge_index, W1, b1, W2, b2, W3, b3, Wr, br, trace=False):
    per_core, C_arr, newid = _preprocess(x, edge_index)
    nc = _get_program(C_arr)

    ws = [np.asarray(w, np.float16) for w in (W1, W2, W3, Wr)]
    bcs = [np.asarray(b, np.float32).reshape(HID, 1) for b in (b1, b2, b3)]
    btr = np.tile(np.asarray(br, np.float32).reshape(1, -1), (BLK, 1))

    in_maps = []
    for k in range(P):
        m = dict(per_core[k])
        for i in range(4):
            m[f"w{i}"] = ws[i]
        for i in range(3):
            m[f"bc{i}"] = bcs[i]
        m["btr"] = btr
        in_maps.append(m)

    res = run_bass_kernel_spmd(nc, in_maps, core_ids=list(range(P)),
                               trace=trace)
    allp = np.concatenate([res.results[k]["probs"] for k in range(P)], axis=0)
    out = allp[newid]
    kernel.last_results = res
    return out


# revision 26
# speedup vs baseline: 1.2457x; 1.0028x over previous
"""GCN (3-layer + readout) on 8 Trainium2 NeuronCores.

Strategy (dst-node sharding, 1D graph parallel):
  - Nodes are sharded across 8 cores (6250/core, padded to 6272 = 49 blocks
    of 128).  Each core aggregates messages for the edges whose dst lands in
    its shard.
  - Everything on the message path is fp16 (the rel-err budget is 2e-2).
  - Per layer: each core transforms its OWN shard (z = h @ W on the PE,
    rows scaled by dinv = deg^-1/2 fused into the Scalar-engine PSUM->SBUF
    copy, so table rows are dinv[src]*z[src]) and the shard tables are
    AllGathered; layer 1 reads the own x shard directly.  The transform
    also keeps a feature-major transpose zT of the own rows, so the
    implicit GCN self-loop becomes h = relu(dinv*(agg + zT) + b) with no
    per-edge work.
  - Edge gathers: one indirect DMA (InstDMACopy SWDGE, int32 row ids, one
    128B fp16 row per partition) per 128-edge chunk.  The per-chunk cost
    (~1.1us Q7 descriptor emission + ~1.3us SDMA drain of 128 random 128B
    HBM reads, overlapped) is the hard floor of this kernel; batched
    InstDMAGatherAnt measures the same ~8ns/row emission rate, so nothing
    amortizes it.  Everything else is arranged to hide under the gather
    stream.  Edges are sorted by src row inside each (core, dst-block)
    bucket for HBM locality.
  - Scatter-add on the TensorEngine with messages stationary:
    psum[64 feat, 128 dst] += msg[128e, 64f]^T @ onehot[128e, 128d], so the
    block aggregate lands feature-major and feeds the next layer's
    transform (lhsT = hT block) with no transposes on the hot path.
  - One-hot matrices are static per graph: precomputed on the host, stored
    e-major in DRAM, and streamed per 16-chunk group with a single
    contiguous HWDGE DMA (4KB per partition) instead of being built on the
    Vector engine.
  - dst-side dinv scale happens in feature-major space via a precomputed
    broadcast tile dinvb[64, PADS] (rank-1 PE matmuls of ones x dinv row).
  - Host-side preprocessing is strictly index/metadata work (edge bucketing,
    padding, degree counting); all float math runs on device.
"""

import numpy as np

from concourse import bacc, bass, mybir, tile
from concourse.bass_utils import run_bass_kernel_spmd

# ---------------------------------------------------------------- constants
P = 8                      # cores
N = 50000                  # nodes
IN_DIM = 128
HID = 64
OUT_DIM = 10
BLK = 128
G = 16                     # chunks per onehot-stream group

F32 = mybir.dt.float32
F16 = mybir.dt.float16
I32 = mybir.dt.int32

SHARD = N // P
NBLK = (SHARD + BLK - 1) // BLK      # 49
PADS = NBLK * BLK                    # 6272
TBL = P * PADS                       # 50176
NFULL = P * NBLK                     # 392


# ------------------------------------------------------------- host prep
def _preprocess(x, edge_index):
    """Bucket edges (incl. one self-edge per node) into per-(core, dst-block)
    128-edge chunks.

    Nodes are bin-packed into the P*NBLK (core, block) bins by in-degree
    (capacity-constrained LPT) so every bin carries ~the same edge count —
    this minimizes the uniform per-block chunk counts, which set the Q7
    gather-instruction floor.
    """
    import heapq

    x = np.asarray(x, np.float32)
    ei = np.asarray(edge_index, np.int64)
    src, dst = ei[0], ei[1]

    degE = np.bincount(dst, minlength=N).astype(np.int64)
    deg = (degE + 1).astype(np.float32)

    NBINS = P * NBLK
    order_n = np.argsort(-degE, kind="stable")
    heap = [(0, b) for b in range(NBINS)]
    heapq.heapify(heap)
    fill = np.zeros(NBINS, np.int64)
    node_bin = np.empty(N, np.int64)
    node_slot = np.empty(N, np.int64)
    for n in order_n:
        while True:
            s, b = heapq.heappop(heap)
            if fill[b] < BLK:
                break
        node_bin[n] = b
        node_slot[n] = fill[b]
        fill[b] += 1
        heapq.heappush(heap, (s + int(degE[n]) + 1, b))

    newid = node_bin * BLK + node_slot          # padded global row of each node

    # self-loops are handled as an explicit dinv^2*z term, not as edges
    all_src, all_dst = src, dst

    rows = newid[all_src]
    owner = node_bin[all_dst] // NBLK
    blk = node_bin[all_dst] % NBLK
    dstl = node_slot[all_dst].astype(np.int64)

    # bucket + in-bucket src sort (HBM locality for the gather descriptors)
    key = owner * NBLK + blk
    order = np.lexsort((rows, key))
    key_s = key[order]
    counts = np.bincount(key_s, minlength=P * NBLK)
    starts = np.concatenate([[0], np.cumsum(counts)[:-1]])
    pos = np.arange(key_s.size) - starts[key_s]

    # per-block chunk count: max over cores (program is core-uniform)
    C_arr = np.maximum(np.ceil(
        counts.reshape(P, NBLK).max(axis=0) / BLK).astype(np.int64), 1)
    base = np.concatenate([[0], np.cumsum(C_arr)[:-1]])
    T = int(C_arr.sum())

    own_s = key_s // NBLK
    blk_s = key_s % NBLK
    slot = base[blk_s] * BLK + pos            # (chunk, lane) within the stream
    flat = own_s * (T * BLK) + slot

    gidx = np.zeros((P, T * BLK), np.int32)
    gidx.reshape(-1)[flat] = rows[order]
    dv = np.full((P, T * BLK), -1, np.int64)
    dv.reshape(-1)[flat] = dstl[order]

    lanes = np.arange(T * BLK)
    x_pad = np.zeros((TBL, IN_DIM), np.float32)
    deg_pad = np.ones((P, PADS), np.float32)
    x_pad[newid] = x
    deg_pad.reshape(-1)[newid] = deg
    xpt = np.ascontiguousarray(x_pad.T.astype(np.float16))          # [128, TBL]
    degp = np.ascontiguousarray(
        deg_pad.reshape(NFULL, BLK).T)                              # [128, 392]

    per_core = []
    for k in range(P):
        oh = np.zeros((BLK, T * BLK), np.float16)
        dvk = dv[k]
        sel = dvk >= 0
        oh[lanes[sel] % BLK, (lanes[sel] // BLK) * BLK + dvk[sel]] = 1.0
        per_core.append(dict(
            xpo=np.ascontiguousarray(xpt[:, k * PADS:(k + 1) * PADS]),
            dego=np.ascontiguousarray(degp[:, k * NBLK:(k + 1) * NBLK]),
            degbt=np.ascontiguousarray(deg_pad[k].reshape(1, PADS)),
            gidx=np.ascontiguousarray(gidx[k].reshape(T, BLK).T),
            ohd=oh,
            iden=np.eye(BLK, dtype=np.float16),
        ))
    return per_core, tuple(int(c) for c in C_arr), newid


# ------------------------------------------------------------- device build
def _build(C_arr):
    T = int(sum(C_arr))
    c_base = [0]
    for c in C_arr[:-1]:
        c_base.append(c_base[-1] + c)

    nc = bacc.Bacc("TRN2", target_bir_lowering=False, debug=False,
                   enable_asserts=False, num_devices=P,
                   dynamic_dma_scratch_size=65536)

    xpo_d = nc.dram_tensor("xpo", [IN_DIM, PADS], F16, kind="ExternalInput").ap()
    dego_d = nc.dram_tensor("dego", [BLK, NBLK], F32, kind="ExternalInput").ap()
    degbt_d = nc.dram_tensor("degbt", [1, PADS], F32, kind="ExternalInput").ap()
    gidx_d = nc.dram_tensor("gidx", [BLK, T], I32, kind="ExternalInput").ap()
    ohd_d = nc.dram_tensor("ohd", [BLK, T * BLK], F16, kind="ExternalInput").ap()
    w_d = [nc.dram_tensor(f"w{i}", [d, HID if i < 3 else OUT_DIM], F16,
                          kind="ExternalInput").ap()
           for i, d in enumerate([IN_DIM, HID, HID, HID])]
    bc_d = [nc.dram_tensor(f"bc{i}", [HID, 1], F32, kind="ExternalInput").ap()
            for i in range(3)]
    btr_d = nc.dram_tensor("btr", [BLK, OUT_DIM], F32, kind="ExternalInput").ap()
    iden_d = nc.dram_tensor("iden", [BLK, BLK], F16, kind="ExternalInput").ap()
    out_d = nc.dram_tensor("probs", [PADS, OUT_DIM], F32, kind="ExternalOutput").ap()

    rg = [list(range(P))]

    with tile.TileContext(nc) as tc:
        with (
            tc.tile_pool(name="const", bufs=1) as cp,
            tc.tile_pool(name="zt", bufs=3) as zp,
            tc.tile_pool(name="zT2", bufs=2) as ztp,
            tc.tile_pool(name="oh", bufs=4) as ohp,
            tc.tile_pool(name="msg", bufs=24) as mp,
            tc.tile_pool(name="cmb", bufs=4) as cb,
            tc.tile_pool(name="fin", bufs=2) as fp,
            tc.tile_pool(name="psz", bufs=2, space="PSUM") as psz,
            tc.tile_pool(name="pstp", bufs=1, space="PSUM") as pstp,
            tc.tile_pool(name="psacc", bufs=3, space="PSUM") as psacc,
            tc.tile_pool(name="pso", bufs=1, space="PSUM") as pso,
            tc.tile_pool(name="dram", bufs=1, space="DRAM") as dp,
        ):
            # ---- constants into SBUF
            w_sb, bc_sb = [], []
            for i in range(4):
                wt = cp.tile(list(w_d[i].shape), F16, tag=f"w{i}", name=f"w{i}")
                nc.sync.dma_start(wt[:], w_d[i])
                w_sb.append(wt)
            for i in range(3):
                bt = cp.tile([HID, 1], F32, tag=f"bc{i}", name=f"bc{i}")
                nc.sync.dma_start(bt[:], bc_d[i])
                bc_sb.append(bt)
            btr_sb = cp.tile([BLK, OUT_DIM], F32, tag="btr")
            nc.sync.dma_start(btr_sb[:], btr_d)
            iden_sb = cp.tile([BLK, BLK], F16, tag="iden")
            nc.sync.dma_start(iden_sb[:], iden_d)
            gidx_sb = cp.tile([BLK, T], I32, tag="gidx")
            nc.sync.dma_start(gidx_sb[:], gidx_d)
            xpo_sb = cp.tile([IN_DIM, PADS], F16, tag="xpo")
            nc.sync.dma_start(xpo_sb[:], xpo_d)

            # dinv = deg^-1/2 in the two layouts we need
            dinvo_sb = cp.tile([BLK, NBLK], F32, tag="dinvo")
            nc.sync.dma_start(dinvo_sb[:], dego_d)
            nc.vector.reciprocal(dinvo_sb[:], dinvo_sb[:])
            nc.scalar.activation(dinvo_sb[:], dinvo_sb[:],
                                 mybir.ActivationFunctionType.Sqrt)
            dinvbt_sb = cp.tile([1, PADS], F32, tag="dinvbt")
            nc.sync.dma_start(dinvbt_sb[:], degbt_d)
            nc.vector.reciprocal(dinvbt_sb[:], dinvbt_sb[:])
            nc.scalar.activation(dinvbt_sb[:], dinvbt_sb[:],
                                 mybir.ActivationFunctionType.Sqrt)

            # dinvb[64, PADS] (f16): dst-side dinv broadcast across the
            # feature partitions, built with rank-1 matmuls ones^T x dinv_row
            ones_sb = cp.tile([1, HID], F32, tag="ones")
            nc.vector.memset(ones_sb[:], 1.0)
            dinvb_sb = cp.tile([HID, PADS], F16, tag="dinvb")
            for i in range(PADS // 448):
                off = i * 448
                ps = pso.tile([HID, 448], F32, tag="bc", name="bc_ps")
                nc.tensor.matmul(ps[:], ones_sb[:], dinvbt_sb[:, off:off + 448],
                                 start=True, stop=True)
                nc.vector.tensor_copy(dinvb_sb[:, off:off + 448], ps[:])

            hT = [cp.tile([HID, PADS], F16, tag=f"h{i}", name=f"h{i}")
                  for i in range(2)]

            TGB = 7

            def transform_layer(src_sb, li):
                """table = AllGather(dinv * (src @ W_li)) — src feature-major.

                Also returns zT[64, PADS], the own-shard table rows
                transposed back to feature-major for the self-loop term."""
                ag_in = dp.tile([PADS, HID], F16, tag=f"agin{li}",
                                name=f"agin{li}")
                zT = ztp.tile([HID, PADS], F16, tag="zT", name="zT")
                for g in range(NBLK // TGB):
                    zg = zp.tile([BLK, TGB * HID], F16, tag="zd", name="zd")
                    for j in range(TGB):
                        b = g * TGB + j
                        z_ps = psz.tile([BLK, HID], F32, tag="z", name="z_ps")
                        nc.tensor.matmul(z_ps[:],
                                         src_sb[:, b * BLK:(b + 1) * BLK],
                                         w_sb[li][:], start=True, stop=True)
                        nc.scalar.activation(zg[:, j * HID:(j + 1) * HID],
                                             z_ps[:],
                                             mybir.ActivationFunctionType.Copy,
                                             scale=dinvo_sb[:, b:b + 1])
                        tp = pstp.tile([HID, BLK], F16, tag="tp", name="tp")
                        nc.tensor.transpose(tp[:],
                                            zg[:, j * HID:(j + 1) * HID],
                                            iden_sb[:])
                        nc.vector.tensor_copy(zT[:, b * BLK:(b + 1) * BLK],
                                              tp[:])
                    nc.sync.dma_start(
                        ag_in[g * TGB * BLK:(g + 1) * TGB * BLK, :].rearrange(
                            "(j p) f -> p j f", p=BLK),
                        zg[:].rearrange("p (j f) -> p j f", f=HID))
                table = dp.tile([TBL, HID], F16, tag=f"tbl{li}",
                                name=f"table{li}", addr_space="Shared")
                nc.gpsimd.collective_compute(
                    "AllGather", mybir.AluOpType.bypass, replica_groups=rg,
                    ins=[ag_in.opt()], outs=[table.opt()])
                return table, zT

            def readout_block(h_ap, b):
                o_ps = pso.tile([BLK, OUT_DIM], F32, tag="o", name="o_ps")
                nc.tensor.matmul(o_ps[:], h_ap, w_sb[3][:], start=True, stop=True)
                logit = fp.tile([BLK, OUT_DIM], F32, tag="logit", name="logit")
                nc.vector.tensor_tensor(logit[:], o_ps[:], btr_sb[:],
                                        mybir.AluOpType.add)
                nmx = fp.tile([BLK, 1], F32, tag="nmx", name="nmx")
                nc.vector.reduce_max(nmx[:], logit[:],
                                     axis=mybir.AxisListType.X, negate=True)
                ex = fp.tile([BLK, OUT_DIM], F32, tag="ex", name="ex")
                ssum = fp.tile([BLK, 1], F32, tag="ssum", name="ssum")
                nc.scalar.activation(ex[:], logit[:],
                                     mybir.ActivationFunctionType.Exp,
                                     bias=nmx[:], accum_out=ssum[:])
                rs = fp.tile([BLK, 1], F32, tag="rs", name="rs")
                nc.vector.reciprocal(rs[:], ssum[:])
                prob = fp.tile([BLK, OUT_DIM], F32, tag="prob", name="prob")
                nc.vector.tensor_scalar(prob[:], ex[:], rs[:], None,
                                        mybir.AluOpType.mult)
                nc.sync.dma_start(out_d[b * BLK:(b + 1) * BLK, :], prob[:])

            def propagate(table, zT, h_out, bc_t, readout=False):
                oh_tiles = {}

                def oh_group(tg):
                    if tg not in oh_tiles:
                        lo = tg * G
                        n = min(T, lo + G) - lo
                        oh = ohp.tile([BLK, G * BLK], F16, tag="oh", name="oh")
                        nc.sync.dma_start(oh[:, :n * BLK],
                                          ohd_d[:, lo * BLK:(lo + n) * BLK])
                        oh_tiles[tg] = oh
                    return oh_tiles[tg]

                for b in range(NBLK):
                    C_b = C_arr[b]
                    agg = psacc.tile([HID, BLK], F32, tag="acc", name="agg")
                    for c in range(C_b):
                        t = c_base[b] + c
                        tg, r = divmod(t, G)
                        oh = oh_group(tg)
                        msg = mp.tile([BLK, HID], F16, tag="msg", name="msg")
                        nc.gpsimd.indirect_dma_start(
                            out=msg[:], out_offset=None, in_=table[:],
                            in_offset=bass.IndirectOffsetOnAxis(
                                ap=gidx_sb[:, t:t + 1], axis=0))
                        nc.tensor.matmul(agg[:], msg[:],
                                         oh[:, r * BLK:(r + 1) * BLK],
                                         start=(c == 0), stop=(c == C_b - 1))
                    sl = slice(b * BLK, (b + 1) * BLK)
                    t1 = cb.tile([HID, BLK], F16, tag="t1", name="t1")
                    nc.vector.tensor_tensor(t1[:], agg[:], zT[:, sl],
                                            mybir.AluOpType.add)
                    tmp = cb.tile([HID, BLK], F16, tag="tmp", name="tmp")
                    nc.vector.tensor_tensor(tmp[:], t1[:], dinvb_sb[:, sl],
                                            mybir.AluOpType.mult)
                    nc.scalar.activation(h_out[:, sl], tmp[:],
                                         mybir.ActivationFunctionType.Relu,
                                         bias=bc_t[:])
                    if readout:
                        readout_block(h_out[:, sl], b)

            # layer 1 transforms the own x shard; layers 2/3 the own h shard
            table, zT = transform_layer(xpo_sb, 0)
            propagate(table, zT, hT[0], bc_sb[0])
            for li in (1, 2):
                table, zT = transform_layer(hT[(li + 1) % 2], li)
                propagate(table, zT, hT[li % 2], bc_sb[li],
                          readout=(li == 2))

    nc.compile()
    return nc


# ------------------------------------------------------------- entry point
_CACHE = {}


def _get_program(C_arr):
    if C_arr not in _CACHE:
        _CACHE[C_arr] = _build(C_arr)
    return _CACHE[C_arr]


def kernel(x, edge_index, W1, b1, W2, b2, W3, b3, Wr, br, trace=False):
    per_core, C_arr, newid = _preprocess(x, edge_index)
    nc = _get_program(C_arr)

    ws = [np.asarray(w, np.float16) for w in (W1, W2, W3, Wr)]
    bcs = [np.asarray(b, np.float32).reshape(HID, 1) for b in (b1, b2, b3)]
    btr = np.tile(np.asarray(br, np.float32).reshape(1, -1), (BLK, 1))

    in_maps = []
    for k in range(P):
        m = dict(per_core[k])
        for i in range(4):
            m[f"w{i}"] = ws[i]
        for i in range(3):
            m[f"bc{i}"] = bcs[i]
        m["btr"] = btr
        in_maps.append(m)

    res = run_bass_kernel_spmd(nc, in_maps, core_ids=list(range(P)),
                               trace=trace)
    allp = np.concatenate([res.results[k]["probs"] for k in range(P)], axis=0)
    out = allp[newid]
    kernel.last_results = res
    return out
